# revision 35
# baseline (speedup 1.0000x reference)
"""ByteContextEncoder Trainium2 kernel.

8-core SPMD sharding: core c handles batch row c//2, sequence half c%2
(T_loc = 1024 tokens). Attention needs full-row K/V -> pair AllGather of the
normed hidden state as an fp8 hi/lo pair ([0,1],[2,3],[4,5],[6,7]); K/V
projections are recomputed per core from the gathered h. Segment mean
pooling via one-hot gather/scatter matmuls; the one segment that can span
the half boundary is fixed with a tiny pair AllGather.

Precision: q/k/v/w1/w2 projections run as 3-term fp8 DoubleRow matmuls
(e4m3 hi + e5m2 lo on both weight and activation; hi@hi + lo@hi + hi@lo),
which matches bf16 accuracy at ~3/4 of the PE cycles. Scores, att@v, wo,
w3 and pooling stay bf16; the residual stream stays f32. Attention exp is
split between the Activation engine (true exp) and the DVE (exp2 bit-trick
into bf16 via int16 bitcast); GPSIMD cannot touch PSUM so it only gets
SBUF-side work (rope adds, pooling scales).
"""

import math

import numpy as np
import ml_dtypes

import concourse.bass as bass
import concourse.mybir as mybir
import concourse.tile as tile

BF16 = mybir.dt.bfloat16
F32 = mybir.dt.float32
FP8 = mybir.dt.float8e4
FP8E5 = mybir.dt.float8e5
I16 = mybir.dt.int16
AX = mybir.AxisListType
ALU = mybir.AluOpType
ACT = mybir.ActivationFunctionType
DR = mybir.MatmulPerfMode.DoubleRow

# model dims (hardcoded per problem spec)
B, T, D, H, L = 4, 2048, 512, 8, 2
FF = 4 * D
HD = D // H
EPS = 1e-6
ALPHA = 0.5

N_CORES = 8

P = 128
TL = T // 2          # tokens per core
NT = TL // P         # 8 local token tiles
KT2 = T // P         # 16 full-row token tiles
DC = D // P          # 4 D chunks
FFC = FF // P        # 16 FF chunks
SEG = 384            # padded segments per core
SC = SEG // P        # 3 segment chunks
QBW = 256            # q-block width for attention
NQB = TL // QBW      # 4 q blocks

# fast-exp (exp(x) ~ bf16 bit trick): bits = x*log2(e)*128 + 16256 - 5.5 + .5
FEXP_SCALE = 184.66496280551604 / 8.0
FEXP_BIAS = 16251.5

# exp slab routing (Pool cannot read PSUM): weighted Act:DVE = 9:7 schedule
def exp_route(slab_idx):
    s = slab_idx % 16
    return "a" if (s * 2) // 3 > ((s - 1) * 2) // 3 or s == 0 else "d"

_SEP = b" \t\n\r.,;:!?()[]{}\"'" + b"+-*/=<>|&^~%@#$\\"
SEP_TABLE = np.zeros(256, dtype=bool)
SEP_TABLE[list(_SEP)] = True


def split_multiwait_drains(nc, max_waits=1):
    """This container's walrus can't encode >1 sync-wait on an instruction;
    hoist extra waits onto single-wait NoOps just before it (same engine, so
    sequencer order preserves the wait-before-execute semantics)."""
    n_patched = 0
    for f in nc.m.functions:
        for bb in f.blocks:
            new_list = []
            changed = False
            for ins in bb.instructions:
                si = ins.sync_info
                if si is not None and si.on_wait and len(si.on_wait) > max_waits:
                    for k, w in enumerate(si.on_wait):
                        nop = mybir.InstNoOp(name=f"{ins.name}-w{k}", ins=[], outs=[])
                        nop.engine = ins.engine
                        nop.sync_info = mybir.SyncInfo(on_wait=[w], on_update=[])
                        new_list.append(nop)
                    ins.sync_info = mybir.SyncInfo(
                        on_wait=[], on_update=list(si.on_update)
                    )
                    changed = True
                    n_patched += 1
                new_list.append(ins)
            if changed:
                bb.instructions = new_list
    return n_patched


def build_program(debug=(), patch=True, stage=5, for_sim=False, apply_fw=False):
    nc = bass.Bass(num_devices=N_CORES)

    # ---------------- DRAM inputs ----------------
    d_emb = nc.dram_tensor("emb", [256, D], BF16, kind="ExternalInput")
    d_ident = nc.dram_tensor("ident", [P, P], BF16, kind="ExternalInput")
    d_obind = nc.dram_tensor("obind", [P, TL], BF16, kind="ExternalInput")
    d_oet = nc.dram_tensor("oet", [256, TL], BF16, kind="ExternalInput")
    d_og = nc.dram_tensor("og", [TL, SEG], BF16, kind="ExternalInput")
    d_otg = nc.dram_tensor("otg", [SEG, TL], BF16, kind="ExternalInput")
    d_icnt = nc.dram_tensor("icnt", [SEG], F32, kind="ExternalInput")
    d_esnd = nc.dram_tensor("esnd", [SEG, P], BF16, kind="ExternalInput")
    d_bm = nc.dram_tensor("bm", [P, NT], BF16, kind="ExternalInput")
    d_fw = nc.dram_tensor("fw", [D], F32, kind="ExternalInput")
    d_cos = nc.dram_tensor("cos", [P, TL], BF16, kind="ExternalInput")
    d_sin = nc.dram_tensor("sin", [P, TL], BF16, kind="ExternalInput")
    d_cosf = nc.dram_tensor("cosf", [P, T], BF16, kind="ExternalInput")
    d_sinf = nc.dram_tensor("sinf", [P, T], BF16, kind="ExternalInput")
    d_wq = nc.dram_tensor("wq", [L, D, D], FP8, kind="ExternalInput")
    d_wqr = nc.dram_tensor("wqr", [L, D, D], FP8, kind="ExternalInput")
    d_wql = nc.dram_tensor("wql", [L, D, D], FP8E5, kind="ExternalInput")
    d_wqrl = nc.dram_tensor("wqrl", [L, D, D], FP8E5, kind="ExternalInput")
    d_wk = nc.dram_tensor("wk", [L, D, D], FP8, kind="ExternalInput")
    d_wkr = nc.dram_tensor("wkr", [L, D, D], FP8, kind="ExternalInput")
    d_wkl = nc.dram_tensor("wkl", [L, D, D], FP8E5, kind="ExternalInput")
    d_wkrl = nc.dram_tensor("wkrl", [L, D, D], FP8E5, kind="ExternalInput")
    d_wv = nc.dram_tensor("wv", [L, D, D], FP8, kind="ExternalInput")
    d_wvl = nc.dram_tensor("wvl", [L, D, D], FP8E5, kind="ExternalInput")
    d_wo = nc.dram_tensor("wo", [L, D, D], BF16, kind="ExternalInput")
    d_w1 = nc.dram_tensor("w1", [L, D, FF], FP8, kind="ExternalInput")
    d_w2 = nc.dram_tensor("w2", [L, D, FF], FP8, kind="ExternalInput")
    d_w1l = nc.dram_tensor("w1l", [L, D, FF], FP8E5, kind="ExternalInput")
    d_w2l = nc.dram_tensor("w2l", [L, D, FF], FP8E5, kind="ExternalInput")
    d_w3 = nc.dram_tensor("w3", [L, FF, D], BF16, kind="ExternalInput")

    d_y = nc.dram_tensor("y", [TL, D], F32, kind="ExternalOutput")
    dbg_out = {}

    def dbg(name, shape, dtype=F32):
        if name in debug:
            dbg_out[name] = nc.dram_tensor(
                "dbg_" + name, shape, dtype, kind="ExternalOutput"
            )
            return dbg_out[name]
        return None

    with tile.TileContext(nc) as tc:
        with (
            tc.tile_pool(name="state", bufs=1) as state,
            tc.tile_pool(name="aux", bufs=1) as aux,
            tc.tile_pool(name="wsm", bufs=2) as wsm,
            tc.tile_pool(name="wff", bufs=2) as wff,
            tc.tile_pool(name="stp", bufs=2) as stp,
            tc.tile_pool(name="tmp", bufs=4) as tmp,
            tc.tile_pool(name="tmp2", bufs=2) as tmp2,
            tc.tile_pool(name="tmp3", bufs=2) as tmp3,
            tc.tile_pool(name="psum", bufs=2, space="PSUM") as psum,
            tc.tile_pool(name="psum_st", bufs=3, space="PSUM") as psum_st,
            tc.tile_pool(name="dram", bufs=1, space="DRAM") as dram,
        ):
            # ---- persistent state ----
            x_sb = state.tile([P, NT, D], F32, tag="x")          # residual
            cos_sb = state.tile([P, TL], BF16, tag="cos")
            sin_sb = state.tile([P, TL], BF16, tag="sin")
            cosf_sb = state.tile([P, T], BF16, tag="cosf")
            sinf_sb = state.tile([P, T], BF16, tag="sinf")
            ident = state.tile([P, P], BF16, tag="ident")
            eps_sb = state.tile([P, 1], F32, tag="eps")
            nc.vector.memset(eps_sb[:], EPS)

            # ---- embedding: x = onehot @ table (inputs loaded first) ----
            embt = aux.tile([P, 2, D], BF16, tag="otg_embt")
            oet = aux.tile([P, 2, TL], BF16, tag="og_oet")
            nc.sync.dma_start(oet[:], d_oet.rearrange("(c p) t -> p c t", p=P))
            nc.sync.dma_start(embt[:], d_emb.rearrange("(c p) d -> p c d", p=P))
            nc.sync.dma_start(ident[:], d_ident[:])
            nc.sync.dma_start(cos_sb[:], d_cos[:])
            nc.sync.dma_start(sin_sb[:], d_sin[:])
            nc.sync.dma_start(cosf_sb[:], d_cosf[:])
            nc.sync.dma_start(sinf_sb[:], d_sinf[:])
            for t in range(NT):
                ps = psum.tile([P, 512], F32, tag="mm")
                for kc in range(2):
                    nc.tensor.matmul(
                        ps[:],
                        oet[:, kc, t * P : (t + 1) * P],
                        embt[:, kc, :],
                        start=(kc == 0),
                        stop=(kc == 1),
                    )
                nc.scalar.copy(x_sb[:, t, :], ps[:])

            def rmsnorm_tile(t, out_tile, out_slice):
                """out = x_sb[:,t,:] * rsqrt(mean(x^2)+eps)  (dtype of out)."""
                xsq = tmp.tile([P, D], BF16, tag="h")
                ssq = tmp.tile([P, 1], F32, tag="ssq")
                nc.scalar.activation(
                    xsq[:], x_sb[:, t, :], ACT.Square, accum_out=ssq[:]
                )
                nc.scalar.activation(
                    ssq[:], ssq[:], ACT.Sqrt, bias=eps_sb[:], scale=1.0 / D
                )
                nc.vector.reciprocal(ssq[:], ssq[:])
                nc.vector.tensor_scalar_mul(out_tile[out_slice], x_sb[:, t, :], ssq[:])

            def norm_transpose(dst8, dst_lo=None):
                """rmsnorm all NT tiles of x, transpose, write [P, DC, TL]
                (+ optional e5m2 low-part for hi/lo split matmuls)."""
                for t in range(NT):
                    h_t = tmp.tile([P, D], BF16, tag="h")
                    rmsnorm_tile(t, h_t, np.s_[:])
                    pt = psum.tile([P, DC, P], BF16, tag="mm")
                    for c in range(DC):
                        nc.tensor.transpose(
                            pt[:, c, :], h_t[:, c * P : (c + 1) * P], ident[:]
                        )
                    tsl = np.s_[:, :, t * P : (t + 1) * P]
                    nc.scalar.copy(dst8[tsl], pt[:])
                    if dst_lo is not None:
                        nc.vector.scalar_tensor_tensor(
                            out=dst_lo[tsl],
                            in0=dst8[tsl],
                            scalar=-1.0,
                            in1=pt[:],
                            op0=ALU.mult,
                            op1=ALU.add,
                        )

            def dr_gemm(out_ps, w_sb, mov, mc, nsl0, nwid, kchunks, tag_extra=""):
                """bf16 gemm into out_ps[P, nwid] f32 (psum bank),
                contracting kchunks*128 via w_sb [P, kc, .] x mov [P, kc, .]."""
                for kc in range(kchunks):
                    nc.tensor.matmul(
                        out_ps[:],
                        w_sb[:, kc, mc * P : (mc + 1) * P],
                        mov[:, kc, nsl0 : nsl0 + nwid],
                        start=(kc == 0),
                        stop=(kc == kchunks - 1),
                    )

            def gemm3(out_ps, w_hi, w_lo, mv_hi, mv_lo, mc, nsl0, nwid):
                """hi/lo split fp8 DoubleRow gemm: w@h ~ whi@hhi + wlo@hhi
                + whi@hlo, accumulated in one psum group (K=512)."""
                first = True
                n_i = nwid // 256
                terms = ((w_hi, mv_hi), (w_lo, mv_hi), (w_hi, mv_lo))
                for nb in range(n_i):
                    for g in range(2):
                        for ti, (wt, mv) in enumerate(terms):
                            nc.tensor.matmul(
                                out_ps[:, nb * 256 : (nb + 1) * 256],
                                wt[:, 2 * g : 2 * g + 2, mc * P : (mc + 1) * P],
                                mv[:, 2 * g : 2 * g + 2,
                                   nsl0 + nb * 256 : nsl0 + (nb + 1) * 256],
                                start=first,
                                stop=(nb == n_i - 1 and g == 1 and ti == 2),
                                perf_mode=DR,
                                skip_group_check=True,
                            )
                            first = False

            # pooling index matrices: trace early so their DMAs overlap layers
            og_e = aux.tile([P, NT, SEG], BF16, tag="og_oet")
            otg_e = aux.tile([P, SC, TL], BF16, tag="otg_embt")
            icnt_e = aux.tile([P, SC, 1], F32, tag="icnt")
            esnd_e = aux.tile([P, SC, P], BF16, tag="esnd")
            obind_e = aux.tile([P, TL], BF16, tag="obind")
            pool_aux = (og_e, otg_e, icnt_e, esnd_e, obind_e)
            nc.sync.dma_start(og_e[:], d_og.rearrange("(c p) s -> p c s", p=P))
            nc.sync.dma_start(otg_e[:], d_otg.rearrange("(c p) t -> p c t", p=P))
            nc.sync.dma_start(
                icnt_e[:], d_icnt.rearrange("(c p) -> p c", p=P)[:, :, None]
            )
            nc.sync.dma_start(esnd_e[:], d_esnd.rearrange("(c p) m -> p c m", p=P))
            bm_e = aux.tile([P, NT], BF16, tag="bm")
            nc.sync.dma_start(bm_e[:], d_bm[:])
            nc.sync.dma_start(obind_e[:], d_obind[:])

            # ================= layers =================
            for l in range(L if stage >= 4 else (1 if stage >= 2 else 0)):
                hT = state.tile([P, DC, TL], FP8, tag="hT")
                hTl = state.tile([P, DC, TL], FP8E5, tag="hTl")
                norm_transpose(hT, hTl)

                # ---- exchange h within pair (full-row h needed for K/V) ----
                bh_in = dram.tile([2 * D * TL], FP8, tag="bkv_in")
                bh_out = dram.tile([4 * D * TL], FP8, tag="bkv_out")
                nc.sync.dma_start(
                    bh_in[: D * TL].rearrange("(c p x) -> p c x", p=P, c=DC), hT[:]
                )
                nc.sync.dma_start(
                    bh_in[D * TL :].rearrange("(c p x) -> p c x", p=P, c=DC),
                    hTl[:].bitcast(FP8),
                )
                nc.gpsimd.collective_compute(
                    "AllGather",
                    ALU.bypass,
                    replica_groups=[[0, 1], [2, 3], [4, 5], [6, 7]],
                    ins=[bh_in[:].opt()],
                    outs=[bh_out[:].opt()],
                )

                # ---- q projection + rope from local h (overlaps collective) ----
                qT = state.tile([P, DC, TL], BF16, tag="h12")
                w_sb = wsm.tile([P, DC, D], FP8, tag="wsm")
                wr_sb = wsm.tile([P, DC, D], FP8, tag="wsm")
                wl_sb = wsm.tile([P, DC, D], FP8E5, tag="wsml")
                wrl_sb = wsm.tile([P, DC, D], FP8E5, tag="wsml")
                nc.sync.dma_start(w_sb[:], d_wq[l].rearrange("(c p) n -> p c n", p=P))
                nc.sync.dma_start(wr_sb[:], d_wqr[l].rearrange("(c p) n -> p c n", p=P))
                nc.sync.dma_start(wl_sb[:], d_wql[l].rearrange("(c p) n -> p c n", p=P))
                nc.sync.dma_start(wrl_sb[:], d_wqrl[l].rearrange("(c p) n -> p c n", p=P))
                for mc in range(DC):
                    for nh in range(2):
                        nsl = np.s_[nh * 512 : (nh + 1) * 512]
                        pa = psum.tile([P, 512], F32, tag="mm")
                        pb = psum_st.tile([P, 4, QBW], F32, tag="st", name="pb0")
                        pbv = pb.rearrange("p a b -> p (a b)")
                        gemm3(pa, w_sb, wl_sb, hT, hTl, mc, nh * 512, 512)
                        gemm3(pbv[:, :512], wr_sb, wrl_sb, hT, hTl, mc, nh * 512, 512)
                        t1 = tmp.tile([P, 512], BF16, tag="rope")
                        t2 = tmp.tile([P, 512], BF16, tag="rope")
                        nc.vector.tensor_mul(t1[:], pa[:], cos_sb[:, nsl])
                        nc.vector.tensor_mul(t2[:], pbv[:, :512], sin_sb[:, nsl])
                        nc.gpsimd.tensor_add(qT[:, mc, nsl], t1[:], t2[:])

                # ---- gathered full-row h (fp8 hi/lo, global order) ----
                hTf = state.tile([P, DC, T], FP8, tag="hTf")
                hTfl = state.tile([P, DC, T], FP8E5, tag="hTfl")
                for r in range(2):
                    nc.sync.dma_start(
                        hTf[:, :, r * TL : (r + 1) * TL],
                        bh_out[2 * r * D * TL :][: D * TL].rearrange(
                            "(c p t) -> p c t", p=P, c=DC
                        ),
                    )
                    nc.sync.dma_start(
                        hTfl[:, :, r * TL : (r + 1) * TL].bitcast(FP8),
                        bh_out[(2 * r + 1) * D * TL :][: D * TL].rearrange(
                            "(c p t) -> p c t", p=P, c=DC
                        ),
                    )

                # ---- k projection + rope, full row -> kTf ----
                kTfA = state.tile([P, 2, T], BF16, tag="kTfA")
                kTfB = state.tile([P, 2, T], BF16, tag="kTfB")
                wk_sb = wsm.tile([P, DC, D], FP8, tag="wsm")
                wkr_sb = wsm.tile([P, DC, D], FP8, tag="wsm")
                wkl_sb = wsm.tile([P, DC, D], FP8E5, tag="wsml")
                wkrl_sb = wsm.tile([P, DC, D], FP8E5, tag="wsml")
                nc.sync.dma_start(wk_sb[:], d_wk[l].rearrange("(c p) n -> p c n", p=P))
                nc.sync.dma_start(
                    wkr_sb[:], d_wkr[l].rearrange("(c p) n -> p c n", p=P)
                )
                nc.sync.dma_start(wkl_sb[:], d_wkl[l].rearrange("(c p) n -> p c n", p=P))
                nc.sync.dma_start(
                    wkrl_sb[:], d_wkrl[l].rearrange("(c p) n -> p c n", p=P)
                )
                for mc in range(DC):
                    for nh in range(4):
                        nsl = np.s_[nh * 512 : (nh + 1) * 512]
                        pa = psum.tile([P, 512], F32, tag="mm")
                        pb = psum_st.tile([P, 4, QBW], F32, tag="st", name="pb0")
                        pbv = pb.rearrange("p a b -> p (a b)")
                        gemm3(pa, wk_sb, wkl_sb, hTf, hTfl, mc, nh * 512, 512)
                        gemm3(pbv[:, :512], wkr_sb, wkrl_sb, hTf, hTfl, mc, nh * 512, 512)
                        ktile = kTfA if mc < 2 else kTfB
                        t1 = tmp.tile([P, 512], BF16, tag="rope")
                        t2 = tmp.tile([P, 512], BF16, tag="rope")
                        nc.vector.tensor_mul(t1[:], pa[:], cosf_sb[:, nsl])
                        nc.vector.tensor_mul(t2[:], pbv[:, :512], sinf_sb[:, nsl])
                        nc.gpsimd.tensor_add(
                            ktile[:, mc % 2, nsl], t1[:], t2[:]
                        )

                # ---- v (tokens layout + ones col), full row -> vf ----
                vf = state.tile([P, KT2, H, HD + 1], BF16, tag="vf")
                wv_sb = wsm.tile([P, DC, D], FP8, tag="wsm")
                wvl_sb = wsm.tile([P, DC, D], FP8E5, tag="wsml")
                nc.sync.dma_start(wv_sb[:], d_wv[l].rearrange("(c p) n -> p c n", p=P))
                nc.sync.dma_start(wvl_sb[:], d_wvl[l].rearrange("(c p) n -> p c n", p=P))
                for t in range(KT2):
                    ps = psum.tile([P, 512], F32, tag="mm")
                    first = True
                    tsl = np.s_[t * P : (t + 1) * P]
                    for nb in range(2):
                        for g in range(2):
                            for ti, (ht, wt) in enumerate(
                                ((hTf, wv_sb), (hTfl, wv_sb), (hTf, wvl_sb))
                            ):
                                nc.tensor.matmul(
                                    ps[:, nb * 256 : (nb + 1) * 256],
                                    ht[:, 2 * g : 2 * g + 2, tsl],
                                    wt[:, 2 * g : 2 * g + 2,
                                       nb * 256 : (nb + 1) * 256],
                                    start=first,
                                    stop=(nb == 1 and g == 1 and ti == 2),
                                    perf_mode=DR,
                                    skip_group_check=True,
                                )
                                first = False
                    nc.scalar.copy(
                        vf[:, t, :, :HD],
                        ps[:].rearrange("p (h d) -> p h d", h=H),
                    )
                    nc.vector.memset(vf[:, t, :, HD], 1.0)

                # ---- attention ----
                if stage < 3:
                    continue
                o_sb = state.tile([P, NT, D], BF16, tag="o_or_xn")
                for h in range(H):
                    po = (h % 2) * HD
                    hc = h // 2
                    for qb in range(NQB):
                        qsl = np.s_[qb * QBW : (qb + 1) * QBW]
                        st_sb = stp.tile([P, KT2, QBW], BF16, tag="st")
                        for k4 in range(KT2 // 4):
                            pst = psum_st.tile([P, 4, QBW], F32, tag="st")
                            for j in range(4):
                                kt = k4 * 4 + j
                                ksrc = kTfA if hc < 2 else kTfB
                                nc.tensor.matmul(
                                    pst[:, j, :],
                                    ksrc[po : po + HD, hc % 2, kt * P : (kt + 1) * P],
                                    qT[po : po + HD, hc, qsl],
                                    start=True,
                                    stop=True,
                                )
                            route = exp_route((h * NQB + qb) * 4 + k4)
                            dst = st_sb[:, k4 * 4 : (k4 + 1) * 4, :]
                            if route == "a":
                                nc.scalar.activation(
                                    dst, pst[:], ACT.Exp, scale=1.0 / math.sqrt(HD)
                                )
                            else:
                                nc.vector.tensor_scalar(
                                    dst.bitcast(I16),
                                    pst[:],
                                    FEXP_SCALE,
                                    FEXP_BIAS,
                                    op0=ALU.mult,
                                    op1=ALU.add,
                                )
                        for qt in range(QBW // P):
                            tix = qb * (QBW // P) + qt
                            pav = psum.tile([P, 512], F32, tag="mm")
                            for kt in range(KT2):
                                nc.tensor.matmul(
                                    pav[:, : HD + 1],
                                    st_sb[:, kt, qt * P : (qt + 1) * P],
                                    vf[:, kt, h, :],
                                    start=(kt == 0),
                                    stop=(kt == KT2 - 1),
                                )
                            rcp = tmp.tile([P, 1], F32, tag="rcp")
                            nc.vector.reciprocal(rcp[:], pav[:, HD : HD + 1])
                            nc.vector.tensor_scalar_mul(
                                o_sb[:, tix, h * HD : (h + 1) * HD],
                                pav[:, :HD],
                                rcp[:],
                            )

                # o -> oT (fp8) -> wo (DR) -> residual
                oT = state.tile([P, DC, TL], BF16, tag="hTfl")
                for t in range(NT):
                    pt = psum.tile([P, DC, P], BF16, tag="mm")
                    for c in range(DC):
                        nc.tensor.transpose(
                            pt[:, c, :], o_sb[:, t, c * P : (c + 1) * P], ident[:]
                        )
                    nc.scalar.copy(oT[:, :, t * P : (t + 1) * P], pt[:])
                wo_sb = wsm.tile([P, DC, D], BF16, tag="wsm")
                nc.sync.dma_start(wo_sb[:], d_wo[l].rearrange("(c p) n -> p c n", p=P))
                for t in range(NT):
                    ps = psum.tile([P, 512], F32, tag="mm")
                    for kc in range(DC):
                        nc.tensor.matmul(
                            ps[:],
                            oT[:, kc, t * P : (t + 1) * P],
                            wo_sb[:, kc, :],
                            start=(kc == 0),
                            stop=(kc == DC - 1),
                        )
                    nc.vector.tensor_add(x_sb[:, t, :], x_sb[:, t, :], ps[:])

                # ---- FFN ----
                if stage < 4:
                    continue
                hT2 = state.tile([P, DC, TL], FP8, tag="hT")
                hT2l = state.tile([P, DC, TL], FP8E5, tag="hTl")
                norm_transpose(hT2, hT2l)

                w3_sb = wff.tile([P, FFC, D], BF16, tag="w3")
                nc.sync.dma_start(w3_sb[:], d_w3[l].rearrange("(c p) n -> p c n", p=P))
                for th in range(2):
                    nsl = np.s_[th * 512 : (th + 1) * 512]
                    h12 = state.tile([P, FFC, 512], BF16,
                                     tag="h12" if th == 0 else "vf")
                    for mc2 in range(FFC // 2):
                        msl = np.s_[:, mc2 * 2 * P : (mc2 + 1) * 2 * P]
                        w1c = wff.tile([P, DC, 2 * P], FP8, tag="wffc")
                        w2c = wff.tile([P, DC, 2 * P], FP8, tag="wffc")
                        w1cl = wff.tile([P, DC, 2 * P], FP8E5, tag="wffcl")
                        w2cl = wff.tile([P, DC, 2 * P], FP8E5, tag="wffcl")
                        nc.sync.dma_start(
                            w1c[:], d_w1[l][msl].rearrange("(c p) n -> p c n", p=P)
                        )
                        nc.sync.dma_start(
                            w2c[:], d_w2[l][msl].rearrange("(c p) n -> p c n", p=P)
                        )
                        nc.sync.dma_start(
                            w1cl[:], d_w1l[l][msl].rearrange("(c p) n -> p c n", p=P)
                        )
                        nc.sync.dma_start(
                            w2cl[:], d_w2l[l][msl].rearrange("(c p) n -> p c n", p=P)
                        )
                        for mi in range(2):
                            mc = mc2 * 2 + mi
                            p1 = psum.tile([P, 512], F32, tag="mm")
                            p2 = psum_st.tile([P, 4, QBW], F32, tag="st", name="p20")
                            p2v = p2.rearrange("p a b -> p (a b)")
                            gemm3(p1, w1c, w1cl, hT2, hT2l, mi, th * 512, 512)
                            gemm3(p2v[:, :512], w2c, w2cl, hT2, hT2l, mi, th * 512, 512)
                            sl = tmp.tile([P, 512], BF16, tag="h")
                            if for_sim:
                                nc.scalar.activation(sl[:], p1[:], ACT.Sigmoid)
                                u = tmp.tile([P, 512], BF16, tag="h")
                                nc.vector.tensor_mul(u[:], p1[:], sl[:])
                                nc.vector.tensor_mul(h12[:, mc, :], p2v[:, :512], u[:])
                            else:
                                nc.scalar.activation(sl[:], p1[:], ACT.Silu)
                                nc.vector.tensor_mul(h12[:, mc, :], p2v[:, :512], sl[:])

                    for t in range(th * 4, th * 4 + 4):
                        ps = psum.tile([P, 512], F32, tag="mm")
                        for kc in range(FFC):
                            nc.tensor.matmul(
                                ps[:],
                                h12[:, kc, (t - th * 4) * P : (t - th * 4 + 1) * P],
                                w3_sb[:, kc, :],
                                start=(kc == 0),
                                stop=(kc == FFC - 1),
                            )
                        nc.vector.tensor_add(x_sb[:, t, :], x_sb[:, t, :], ps[:])

            # ================= final norm + segment pooling =================
            xn = state.tile([P, NT, D], BF16, tag="o_or_xn")
            for t in range(NT):
                rmsnorm_tile(t, xn, np.s_[:, t, :])

            if stage < 5:
                for t in range(NT):
                    xf = tmp3.tile([P, D], F32, tag="f32t")
                    nc.vector.tensor_copy(xf[:], xn[:, t, :])
                    nc.sync.dma_start(
                        d_y.rearrange("(t p) d -> t p d", p=P)[t], xf[:]
                    )
                og = None
            if stage >= 5:
                og, otg, icnt, esnd, obind = pool_aux
                if apply_fw:
                    fw_bc = aux.tile([P, D], F32, tag="fw_bc")
                    nc.sync.dma_start(
                        fw_bc[:],
                        bass.AP(tensor=d_fw, offset=0, ap=[[0, P], [1, D]]),
                    )

                # boundary partial straight from xn (kicks the pair exchange
                # without waiting for the full segment-sum)
                bseg_in = dram.tile([D], F32, tag="bseg_in")
                bseg_out = dram.tile([2, D], F32, tag="bseg_out")
                pex = psum.tile([P, 512], F32, tag="mm")
                for kt in range(NT):
                    nc.tensor.matmul(
                        pex[:1, :],
                        bm_e[:, kt : kt + 1],
                        xn[:, kt, :],
                        start=(kt == 0),
                        stop=(kt == NT - 1),
                    )
                bpart = tmp2.tile([1, D], F32, tag="bx")
                nc.vector.tensor_copy(bpart[:], pex[:1, :])
                nc.sync.dma_start(bseg_in[:].rearrange("(a d) -> a d", a=1), bpart[:])
                nc.gpsimd.collective_compute(
                    "AllGather",
                    ALU.bypass,
                    replica_groups=[[0, 1], [2, 3], [4, 5], [6, 7]],
                    ins=[bseg_in[:].opt()],
                    outs=[bseg_out[:].opt()],
                )

                segsum_bf = aux.tile([P, SC, D], BF16, tag="segsum_bf")
                for mc in range(SC):
                    ps = psum.tile([P, 512], F32, tag="mm")
                    for kt in range(NT):
                        nc.tensor.matmul(
                            ps[:],
                            og[:, kt, mc * P : (mc + 1) * P],
                            xn[:, kt, :],
                            start=(kt == 0),
                            stop=(kt == NT - 1),
                        )
                    nc.scalar.copy(segsum_bf[:, mc, :], ps[:])

                bg = tmp2.tile([P, D], F32, tag="bx")
                nc.vector.memset(bg[:], 0.0)
                nc.sync.dma_start(bg[:2, :], bseg_out[:])

                # seg_mean (no boundary fix) = my_partial * icnt; the partner
                # contribution is added as a rank-1 correction matmul below so
                # the main scatter does not wait on the collective.
                segmean = aux.tile([P, SC, D], BF16, tag="segmean")
                for mc in range(SC):
                    nc.gpsimd.tensor_scalar_mul(
                        segmean[:, mc, :], segsum_bf[:, mc, :], icnt[:, mc, :]
                    )
                bgfix = tmp.tile([P, D], BF16, tag="h")
                nc.vector.tensor_copy(bgfix[:], bg[:])

                # out = 0.5*xn + scatter(segmean) + obind.T @ partner_partial
                for t in range(NT):
                    ps = psum.tile([P, 512], F32, tag="mm")
                    for kc in range(SC):
                        nc.tensor.matmul(
                            ps[:],
                            otg[:, kc, t * P : (t + 1) * P],
                            segmean[:, kc, :],
                            start=(kc == 0),
                            stop=False,
                        )
                    nc.tensor.matmul(
                        ps[:],
                        obind[:, t * P : (t + 1) * P],
                        bgfix[:],
                        start=False,
                        stop=True,
                    )
                    out_t = tmp3.tile([P, D], F32, tag="f32t")
                    nc.vector.scalar_tensor_tensor(
                        out=out_t[:],
                        in0=xn[:, t, :],
                        scalar=ALPHA,
                        in1=ps[:],
                        op0=ALU.mult,
                        op1=ALU.add,
                    )
                    if apply_fw:
                        nc.vector.tensor_mul(out_t[:], out_t[:], fw_bc[:])
                    nc.sync.dma_start(
                        d_y.rearrange("(t p) d -> t p d", p=P)[t], out_t[:]
                    )

            # ---- debug taps ----
            if "x0" in debug:
                nc.sync.dma_start(
                    dbg_out["x0"].rearrange("(t p) d -> t p d", p=P)[:], x_sb[:]
                )

    if patch:
        split_multiwait_drains(nc)
    return nc


# ----------------------------------------------------------------------------
# host side
# ----------------------------------------------------------------------------


def _rot_cols(w):
    """Columns permuted/negated so (h @ w_rot) == rotate_half(h @ w)."""
    wr = np.empty_like(w)
    for hb in range(0, D, HD):
        wr[:, hb : hb + HD // 2] = -w[:, hb + HD // 2 : hb + HD]
        wr[:, hb + HD // 2 : hb + HD] = w[:, hb : hb + HD // 2]
    return wr


def _to_bf16(a):
    return np.asarray(a, dtype=np.float32).astype(ml_dtypes.bfloat16)


def _to_fp8(a):
    return np.asarray(a, dtype=np.float32).astype(ml_dtypes.float8_e4m3)


def _hi_lo(a):
    a = np.asarray(a, dtype=np.float32)
    hi = a.astype(ml_dtypes.float8_e4m3)
    lo = (a - hi.astype(np.float32)).astype(ml_dtypes.float8_e5m2)
    return hi, lo


def host_prep(inputs):
    tokens = np.clip(np.asarray(inputs["tokens"]), 0, 255).astype(np.int64)
    emb = np.asarray(inputs["embed_table"], np.float32)
    attn_w = np.asarray(inputs["attn_norm_w"], np.float32)
    ffn_w = np.asarray(inputs["ffn_norm_w"], np.float32)
    fin_w = np.asarray(inputs["final_norm_w"], np.float32)
    wq = np.asarray(inputs["wq"], np.float32) * attn_w[:, :, None]
    wk = np.asarray(inputs["wk"], np.float32) * attn_w[:, :, None]
    wv = np.asarray(inputs["wv"], np.float32) * attn_w[:, :, None]
    wo = np.asarray(inputs["wo"], np.float32)
    w1 = np.asarray(inputs["w1"], np.float32) * ffn_w[:, :, None]
    w2 = np.asarray(inputs["w2"], np.float32) * ffn_w[:, :, None]
    w3 = np.asarray(inputs["w3"], np.float32)

    wqr = np.stack([_rot_cols(wq[l]) for l in range(L)])
    wkr = np.stack([_rot_cols(wk[l]) for l in range(L)])

    wq_h, wq_l = _hi_lo(wq)
    wqr_h, wqr_l = _hi_lo(wqr)
    wk_h, wk_l = _hi_lo(wk)
    wkr_h, wkr_l = _hi_lo(wkr)
    wv_h, wv_l = _hi_lo(wv)
    w1_h, w1_l = _hi_lo(w1)
    w2_h, w2_l = _hi_lo(w2)
    shared = {
        "fw": fin_w,
        "ident": _to_bf16(np.eye(P, dtype=np.float32)),
        "emb": _to_bf16(emb),
        "wq": wq_h, "wql": wq_l,
        "wqr": wqr_h, "wqrl": wqr_l,
        "wk": wk_h, "wkl": wk_l,
        "wkr": wkr_h, "wkrl": wkr_l,
        "wv": wv_h, "wvl": wv_l,
        "wo": _to_bf16(wo),
        "w1": w1_h, "w1l": w1_l,
        "w2": w2_h, "w2l": w2_l,
        "w3": _to_bf16(w3),
    }

    # rope tables (rows: 2 head-blocks of 64; same pattern for every head pair)
    inv = 1.0 / (10000.0 ** (np.arange(0, HD, 2, dtype=np.float64) / HD))  # (32,)
    in_maps = []
    for c in range(N_CORES):
        b, half = c // 2, c % 2
        tok = tokens[b, half * TL : (half + 1) * TL]
        posf = np.arange(T, dtype=np.float64)
        ff = posf[None, :] * inv[:, None]  # (32, T)
        cos32, sin32 = np.cos(ff), np.sin(ff)
        cos64 = np.concatenate([cos32, cos32], 0)  # (64, T)
        sin64 = np.concatenate([sin32, sin32], 0)
        cosF = np.concatenate([cos64, cos64], 0)  # (128, T)
        sinF = np.concatenate([sin64, sin64], 0)
        cosT = cosF[:, half * TL : (half + 1) * TL]
        sinT = sinF[:, half * TL : (half + 1) * TL]

        oet = np.zeros((256, TL), np.float32)
        oet[tok, np.arange(TL)] = 1.0

        # segments
        is_sep = SEP_TABLE[tokens[b]]
        seg = np.cumsum(is_sep.astype(np.int64))  # inclusive, full row
        cnt = np.bincount(seg, minlength=seg[-1] + 1).astype(np.float64)
        ids = seg[half * TL : (half + 1) * TL]
        base = ids[0]
        loc = ids - base
        S_loc = int(loc[-1]) + 1
        assert S_loc <= SEG, f"too many segments {S_loc}"
        og = np.zeros((TL, SEG), np.float32)
        og[np.arange(TL), loc] = 1.0
        icnt = np.ones(SEG, np.float64)
        icnt[:S_loc] = 0.5 / np.maximum(cnt[base : base + S_loc], 1.0)
        esnd = np.zeros(SEG, np.float32)
        obind = np.zeros((P, TL), np.float32)
        bmv = np.zeros(TL, np.float32)
        if seg[TL - 1] == seg[TL]:  # a segment spans the half boundary
            sb_loc = int(seg[TL] - base) if half == 1 else int(seg[TL - 1] - base)
            esnd[sb_loc] = 1.0
            s_b = sb_loc + base
            obind[1 - half, :] = (ids == s_b) * (0.5 / max(cnt[s_b], 1.0))
            bmv = (ids == s_b).astype(np.float32)

        in_maps.append(
            dict(
                shared,
                oet=_to_bf16(oet),
                og=_to_bf16(og),
                otg=_to_bf16(og.T.copy()),
                icnt=icnt.astype(np.float32),
                obind=_to_bf16(obind),
                esnd=_to_bf16(np.repeat(esnd[:, None], P, 1)),
                bm=_to_bf16(bmv.reshape(NT, P).T.copy()),
                cos=_to_bf16(cosT),
                sin=_to_bf16(sinT),
                cosf=_to_bf16(cosF),
                sinf=_to_bf16(sinF),
            )
        )
    return in_maps


class Runner:
    """Compile once; keep inputs device-resident; re-upload only changed data."""

    def __init__(self, nc):
        import jax
        import jax.numpy as jnp
        from jax.experimental.shard_map import shard_map
        from jax.sharding import Mesh, PartitionSpec
        import concourse.mybir as mybir_
        from concourse import bass2jax

        bass2jax.install_neuronx_cc_hook()
        self.jax = jax
        self.nc = nc
        in_names, out_names, out_avals, zero_outs = [], [], [], []
        for alloc in nc.m.functions[0].allocations:
            if not isinstance(mybir_.MemoryLocationSet, type) or not isinstance(
                alloc, mybir_.MemoryLocationSet
            ):
                continue
            name = alloc.memorylocations[0].name
            if alloc.kind == "ExternalInput":
                if nc.partition_id_tensor is None or name != nc.partition_id_tensor.name:
                    in_names.append(name)
            elif alloc.kind == "ExternalOutput":
                shape = tuple(alloc.tensor_shape)
                dtype = mybir_.dt.np(alloc.dtype)
                out_names.append(name)
                out_avals.append(jax.core.ShapedArray(shape, dtype))
                zero_outs.append(np.zeros(shape, dtype))
        self.n_params = len(in_names)
        self.in_names = list(in_names)
        self.out_names = out_names
        all_in_names = in_names + out_names
        partition_name = nc.partition_id_tensor.name if nc.partition_id_tensor else None
        if partition_name is not None:
            all_in_names = all_in_names + [partition_name]

        def _body(*args):
            operands = list(args)
            if partition_name is not None:
                operands.append(bass2jax.partition_id_tensor())
            outs = bass2jax._bass_exec_p.bind(
                *operands,
                out_avals=tuple(out_avals),
                in_names=tuple(all_in_names),
                out_names=tuple(out_names),
                lowering_input_output_aliases=(),
                sim_require_finite=True,
                sim_require_nnan=True,
                nc=nc,
            )
            return tuple(outs)

        devices = jax.devices()[:N_CORES]
        mesh = Mesh(np.asarray(devices), ("core",))
        n_in = self.n_params + len(out_names)
        self.sharded = jax.jit(
            shard_map(
                _body,
                mesh=mesh,
                in_specs=(PartitionSpec("core"),) * n_in,
                out_specs=(PartitionSpec("core"),) * len(out_names),
                check_rep=False,
            ),
            keep_unused=True,
        )
        self.mesh = mesh
        self.zero_outs = zero_outs
        self._dev_zero = None
        self._cache_np = {}
        self._cache_dev = {}

    def _put(self, name, arrs):
        """Concat per-core numpy arrays and put sharded on device (cached)."""
        import jax
        from jax.sharding import NamedSharding, PartitionSpec

        cached = self._cache_np.get(name)
        if cached is not None and all(
            a is b or (a.shape == b.shape and np.array_equal(a, b))
            for a, b in zip(cached, arrs)
        ):
            return self._cache_dev[name]
        glob = np.concatenate([np.asarray(a) for a in arrs], axis=0)
        dev = jax.device_put(glob, NamedSharding(self.mesh, PartitionSpec("core")))
        self._cache_np[name] = [np.asarray(a) for a in arrs]
        self._cache_dev[name] = dev
        return dev

    def __call__(self, in_maps):
        import jax
        from jax.sharding import NamedSharding, PartitionSpec

        args = [
            self._put(name, [m[name] for m in in_maps]) for name in self.in_names
        ]
        if self._dev_zero is None:
            self._dev_zero = [
                jax.device_put(
                    np.zeros((N_CORES * z.shape[0], *z.shape[1:]), z.dtype),
                    NamedSharding(self.mesh, PartitionSpec("core")),
                )
                for z in self.zero_outs
            ]
        outs = self.sharded(*args, *self._dev_zero)
        outs = [np.asarray(o) for o in outs]
        return {
            name: outs[i].reshape(N_CORES, *self.zero_outs[i].shape)
            for i, name in enumerate(self.out_names)
        }


_RUNNER = None
_RUNNER_FLAGS = None


def _get_runner(apply_fw=False):
    global _RUNNER, _RUNNER_FLAGS
    if _RUNNER is None or _RUNNER_FLAGS != (apply_fw,):
        nc = build_program(apply_fw=apply_fw)
        _RUNNER = Runner(nc)
        _RUNNER_FLAGS = (apply_fw,)
    return _RUNNER


def kernel(**inputs):
    apply_fw = not np.allclose(np.asarray(inputs["final_norm_w"]), 1.0)
    runner = _get_runner(apply_fw=apply_fw)
    in_maps = host_prep(inputs)
    res = runner(in_maps)
    y = res["y"]
    out = np.zeros((B, T, D), np.float32)
    for c in range(N_CORES):
        b, half = c // 2, c % 2
        out[b, half * TL : (half + 1) * TL, :] = y[c]
    return out
